# revision 1
# baseline (speedup 1.0000x reference)
# Triplet-margin loss kernel for Trainium2 (Bass/Tile), batch-sharded
# across 8 NeuronCores.
#
# reference math (torch F.pairwise_distance semantics):
#   d_ap[b,p] = || anc[b] - pos[b,p] + eps ||_2
#   d_an[b,n] = || anc[b] - neg[b,n] + eps ||_2
#   loss = mean_{b,p,n} max(d_ap[b,p] - d_an[b,n] + margin, 0)
#
# With a' = anc + eps, each of the 24 distance columns ("slices") per
# 128-row batch tile is computed by one of three engine paths, chosen to
# balance busy time across Vector/Scalar/GpSimd (DMA is the roofline):
#   T1: d2 = ||a'||^2 - 2 a'.x + ||x||^2  — dot on DVE (scalar_tensor_
#       tensor w/ fp32 accum), norm on ACT (activation Square w/ accum)
#   T3: u = x - a' on GpSimd, d2 = sum u^2 on ACT
#   T4: u = x - a' on GpSimd, d2 = sum u^2 on DVE
# T3/T4 write sum(u^2) into the nrm column and the dot column is
# prefilled with ||a'||^2/2, so the shared combine
#   d = sqrt((nrm - 2*dot) + ||a'||^2)
# yields the right value for all three paths.
# The (p,n) combination uses scalar_tensor_tensor(subtract, min 0, accum)
# which yields -sum_n relu(d_ap - d_an + margin) per (b,p) in one op.
# Each core returns per-partition partial sums [128, 2]; the host sums
# and scales.

import numpy as np

import concourse.bacc as bacc
import concourse.mybir as mybir
import concourse.tile as tile
from concourse import bass_utils

B, Z = 2048, 1024
NUM_POS, NUM_NEG = 8, 16
NJ = NUM_POS + NUM_NEG
MARGIN, EPS = 1.0, 1e-6
N_CORES = 8
BL = B // N_CORES  # 256 rows of anc per core
P = 128
NT = BL // P  # 2 batch-tiles per core
CH = 2  # z-slices per DMA chunk
CHW = CH * Z
NCHUNK = NJ // CH  # 12 chunks per tile

# slice-type split per tile: jj < T1_END -> T1 (dot on DVE + norm on ACT);
# jj >= T1_END -> W (u = x - 2a' on Pool, then sum x*u on DVE, which equals
# d^2 - ||a'||^2 so no ACT work at all).
T1_END = 20

F32 = mybir.dt.float32
AF = mybir.ActivationFunctionType
OP = mybir.AluOpType


def _emit(tc, nc, anc, pos, neg, out):
    v = nc.vector
    act = nc.scalar
    gp = nc.gpsimd
    pos2 = pos.rearrange("(b j) z -> b (j z)", j=NUM_POS)  # [BL, 8*Z]
    neg2 = neg.rearrange("(b j) z -> b (j z)", j=NUM_NEG)  # [BL, 16*Z]
    # issue Pool-owned chunks (high jj) first so the slowest per-slice
    # engine starts early; the T1 chunks (DVE+ACT) drain the tail.
    CHUNK_ORDER = list(range(NCHUNK // 2, NCHUNK)) + list(range(NCHUNK // 2))
    with (
        tc.tile_pool(name="xp", bufs=8) as xp,
        tc.tile_pool(name="up", bufs=6) as up,
        tc.tile_pool(name="apool", bufs=2) as apool,
        tc.tile_pool(name="scp", bufs=1) as scp,
        tc.tile_pool(name="smp", bufs=2) as smp,
        tc.tile_pool(name="opool", bufs=1) as opool,
    ):
        osb = opool.tile([P, NT], F32, name="osb")
        dve_scr = scp.tile([P, Z], F32, name="dve_scr")
        act_scr = scp.tile([P, Z], F32, name="act_scr")
        ts_out = scp.tile([P, NUM_NEG], F32, name="ts_out")
        eps_t = opool.tile([P, 1], F32, name="eps_t")
        v.memset(eps_t[:, :], EPS)
        eps2_t = opool.tile([P, 1], F32, name="eps2_t")
        v.memset(eps2_t[:, :], 2.0 * EPS)
        zero_n = opool.tile([P, NUM_NEG], F32, name="zero_n")
        v.memset(zero_n[:, :], 0.0)
        for t in range(NT):
            b0 = t * P
            anc_in = apool.tile([P, Z], F32, name="anc_in")
            aprime = apool.tile([P, Z], F32, name="aprime")
            a_nrm = smp.tile([P, 1], F32, name="a_nrm")
            dot = smp.tile([P, NJ], F32, name="dot")
            nrm = smp.tile([P, NJ], F32, name="nrm")
            d2c = smp.tile([P, NJ], F32, name="d2c")
            dt_ = smp.tile([P, NJ], F32, name="dt_")
            s_m = smp.tile([P, NUM_POS], F32, name="s_m")
            lp = smp.tile([P, NUM_POS], F32, name="lp")

            a2 = apool.tile([P, Z], F32, name="a2")
            nc.sync.dma_start(anc_in[:, :], anc[b0 : b0 + P, :])
            act.activation(
                aprime[:, :], anc_in[:, :], AF.Identity, bias=eps_t[:, 0:1], scale=1.0
            )
            # a2 = 2*(anc + eps), the subtrahend of the Pool w-path
            act.activation(
                a2[:, :], anc_in[:, :], AF.Identity, bias=eps2_t[:, 0:1], scale=2.0
            )
            act.activation(
                act_scr[:, :], aprime[:, :], AF.Square, accum_out=a_nrm[:, 0:1]
            )
            # W slices write d^2 - ||a'||^2 into their nrm column, so their
            # dot column must contribute nothing to the shared combine.
            v.memset(dot[:, T1_END:NJ], 0.0)

            chunks = {}
            for c in CHUNK_ORDER:
                xt = xp.tile([P, CHW], F32, name="xt")
                if c < NUM_POS // CH:
                    src = pos2[b0 : b0 + P, c * CHW : (c + 1) * CHW]
                else:
                    cn = c - NUM_POS // CH
                    src = neg2[b0 : b0 + P, cn * CHW : (cn + 1) * CHW]
                nc.sync.dma_start(xt[:, :], src)
                chunks[c] = xt

            for c in CHUNK_ORDER:
                xt = chunks[c]
                for q in range(CH):
                    jj = c * CH + q
                    xs = xt[:, q * Z : (q + 1) * Z]
                    if jj < T1_END:
                        v.scalar_tensor_tensor(
                            out=dve_scr[:, :],
                            in0=xs,
                            scalar=1.0,
                            in1=aprime[:, :],
                            op0=OP.bypass,
                            op1=OP.mult,
                            accum_out=dot[:, jj : jj + 1],
                        )
                        act.activation(
                            act_scr[:, :], xs, AF.Square, accum_out=nrm[:, jj : jj + 1]
                        )
                    else:
                        ut = up.tile([P, Z], F32, name="ut")
                        gp.tensor_tensor(
                            out=ut[:, :], in0=xs, in1=a2[:, :], op=OP.subtract
                        )
                        # sum x*(x - 2a') = ||x||^2 - 2 a'.x = d^2 - ||a'||^2
                        v.scalar_tensor_tensor(
                            out=dve_scr[:, :],
                            in0=xs,
                            scalar=1.0,
                            in1=ut[:, :],
                            op0=OP.bypass,
                            op1=OP.mult,
                            accum_out=nrm[:, jj : jj + 1],
                        )

            # d2c = nrm - 2*dot ; d = sqrt(d2c + ||a'||^2)
            v.scalar_tensor_tensor(
                out=d2c[:, :],
                in0=dot[:, :],
                scalar=-2.0,
                in1=nrm[:, :],
                op0=OP.mult,
                op1=OP.add,
            )
            act.activation(
                dt_[:, :], d2c[:, :], AF.Sqrt, bias=a_nrm[:, 0:1], scale=1.0
            )
            # s = d_ap + margin
            v.tensor_scalar_add(s_m[:, :], dt_[:, 0:NUM_POS], MARGIN)
            # lp[:,p] = sum_n min(d_an - s_p, 0) = -sum_n relu(s_p - d_an)
            for p_i in range(NUM_POS):
                v.scalar_tensor_tensor(
                    out=ts_out[:, :],
                    in0=dt_[:, NUM_POS:NJ],
                    scalar=s_m[:, p_i : p_i + 1],
                    in1=zero_n[:, :],
                    op0=OP.subtract,
                    op1=OP.min,
                    accum_out=lp[:, p_i : p_i + 1],
                )
            v.reduce_sum(osb[:, t : t + 1], lp[:, :], axis=mybir.AxisListType.X)
        nc.sync.dma_start(out[:, :], osb[:, :])


_NC_CACHE = None


def build():
    global _NC_CACHE
    if _NC_CACHE is None:
        nc = bacc.Bacc(
            "TRN2", target_bir_lowering=False, debug=False, num_devices=N_CORES
        )
        anc = nc.dram_tensor("anc", (BL, Z), F32, kind="ExternalInput").ap()
        pos = nc.dram_tensor("pos", (BL * NUM_POS, Z), F32, kind="ExternalInput").ap()
        neg = nc.dram_tensor("neg", (BL * NUM_NEG, Z), F32, kind="ExternalInput").ap()
        out = nc.dram_tensor("out", (P, NT), F32, kind="ExternalOutput").ap()
        with tile.TileContext(nc) as tc:
            _emit(tc, nc, anc, pos, neg, out)
        nc.compile()
        _NC_CACHE = nc
    return _NC_CACHE


def make_in_maps(anc_embedding, pos_embedding, neg_embedding):
    anc_embedding = np.asarray(anc_embedding, dtype=np.float32)
    pos_embedding = np.asarray(pos_embedding, dtype=np.float32)
    neg_embedding = np.asarray(neg_embedding, dtype=np.float32)
    in_maps = []
    for c in range(N_CORES):
        in_maps.append(
            {
                "anc": np.ascontiguousarray(anc_embedding[c * BL : (c + 1) * BL]),
                "pos": np.ascontiguousarray(
                    pos_embedding[c * BL * NUM_POS : (c + 1) * BL * NUM_POS]
                ),
                "neg": np.ascontiguousarray(
                    neg_embedding[c * BL * NUM_NEG : (c + 1) * BL * NUM_NEG]
                ),
            }
        )
    return in_maps


def combine(outs):
    # outs: list of [P, NT] per-core partial sums of min(d_an - s, 0)
    total = sum(o.astype(np.float64).sum() for o in outs)
    return np.float32(-total / (B * NUM_POS * NUM_NEG))


def kernel(anc_embedding, pos_embedding, neg_embedding):
    nc = build()
    in_maps = make_in_maps(anc_embedding, pos_embedding, neg_embedding)
    res = bass_utils.run_bass_kernel_spmd(nc, in_maps, core_ids=list(range(N_CORES)))
    return combine([r["out"] for r in res.results])

